# revision 74
# baseline (speedup 1.0000x reference)
"""Causal multi-head attention with RoPE on 8 Trainium2 NeuronCores.

Problem: B=2, L=2048, D_MODEL=1024, N_HEADS=16, D_K=64, theta=10000.
Sharding: data parallel on batch (2) x tensor parallel on heads (4 groups of
4 heads) = 8 cores. Each core computes its 4 heads' attention plus a partial
output projection; partials are summed on the host (Megatron row-parallel).

Per-core device design (v4):
- Q/K live head-contiguous in bf16: each head owns 64 partitions, with its
  RoPE top (even) / bot (odd) dims interleaved in 16-blocks so the rotation
  partner is always p^16 within a 32-quadrant.  RoPE is then 2 full-width
  DVE muls (cos / sign-folded sin) + one stream_shuffle(+-16) + one bf16
  Pool-engine add.  Scores become a SINGLE K=64 matmul per head-tile (the
  cost model charges a matmul by its moving size regardless of K, so a K=32
  top/bot pair would pay 2x), with causal column-slicing on diagonal tiles.
- PSUM: s0/s1 are 2-bank strips (2 heads of scoresT each), a0..a3 1-bank AV
  accumulators (an appended ones-column on V gives the softmax denominator).
  Projections use only s0/s1 (QA+QB / KA+KB / V0..V3 share single 1024-wide
  instances; the two startup V projections borrow the still-untouched av
  tags), so attention's AV tags never WAR-block on the projection pipeline.
- Normalization is PE-free: reciprocal (DVE) -> partition_broadcast (Pool)
  -> scale-copy into ho (DVE).  Inputs arrive in tile-major host layouts so
  each logical group is one contiguous DMA (HWDGE costs a fixed ~625ns per
  instruction); the output is written bf16 and upcast host-side.
- Per chunk c the emission interleaves engines so their queues stay in
  need-order: att(c) | projQ(c+2) mms+muls | recips(c) | projK mms |
  projV mms | homuls(c) | ropeK muls | v-copies | outproj(c) |
  pre-scores(c+1) (AV deferred) | rope combines(c+2).  The prefetched
  score tiles keep PE fed where attention is exp(ACT)-bound.
"""
import numpy as np
import ml_dtypes
from contextlib import ExitStack

import concourse.bacc as bacc
import concourse.bass as bass
import concourse.mybir as mybir
import concourse.tile as tile
from concourse._compat import with_exitstack
from concourse.bass_utils import run_bass_kernel_spmd

F32 = mybir.dt.float32
F32R = mybir.dt.float32r
BF16 = mybir.dt.bfloat16

B, L, DM, NH, DK = 2, 2048, 1024, 16, 64
HPC = 4              # heads per core
THETA = 10000.0
CH = 512             # q/l chunk
NT = L // 128        # 16 kv tiles
NCH = L // CH        # 4 chunks

_cache = {}

# stream_shuffle mask: swap 16-blocks within each 32-quadrant (p <-> p^16)
_SWAP16 = list(range(16, 32)) + list(range(16))


@with_exitstack
def _attn_kernel(ctx: ExitStack, tc: tile.TileContext, outs, ins):
    nc = tc.nc
    xt, wq, wv, wo = ins["xt"], ins["wq"], ins["wv"], ins["wo"]
    cs, sn = ins["cs"], ins["sn"]
    out = outs["out"]
    AF = mybir.ActivationFunctionType

    consts = ctx.enter_context(tc.tile_pool(name="consts", bufs=1))
    persist = ctx.enter_context(tc.tile_pool(name="persist", bufs=1))
    ps = ctx.enter_context(tc.tile_pool(name="ps", bufs=1, space="PSUM"))
    ropet = ctx.enter_context(tc.tile_pool(name="ropet", bufs=6))
    epool = ctx.enter_context(tc.tile_pool(name="epool", bufs=18))
    ipool = ctx.enter_context(tc.tile_pool(name="ipool", bufs=2))
    opool = ctx.enter_context(tc.tile_pool(name="opool", bufs=4))

    # ---- weights / inputs ----
    # xt is loaded as full [128, L] rows: one descriptor per partition for
    # the whole row keeps the (descriptor-bound) HWDGE cost per byte low.
    # wq and xt are interleaved per-d so the d-chain starts early.
    # All inputs arrive in tile-major host layouts so each logical group is
    # ONE contiguous DMA (HWDGE cost is a fixed ~625ns per instruction).
    wqh = []
    xt0h = []
    for piece in range(4):
        t_wqh = consts.tile([128, 2 * 512], BF16, tag=f"wq{piece}")
        nc.sync.dma_start(t_wqh, wq[:, 1024 * piece:1024 * (piece + 1)])
        wqh.append(t_wqh)
        t_x0 = consts.tile([128, 2 * CH], BF16, tag=f"xt0_{piece}",
                           name="t_x0")
        nc.sync.dma_start(t_x0, xt[:, 2 * CH * piece:2 * CH * (piece + 1)])
        xt0h.append((t_x0, 2 * piece))

    def wq_sl(d, lo, hi):
        return wqh[d // 2][:, 512 * (d % 2) + lo:512 * (d % 2) + hi]

    xt_sb = [None] * NCH    # [chunk] -> list of (tile, d-offset) halves
    cs_sb = [None] * NCH
    sn_sb = [None] * NCH

    def _load_cssn(c):
        t_cs = consts.tile([128, CH], F32, tag=f"cs{c}", name="t_cs")
        nc.sync.dma_start(t_cs, cs[:, CH * c:CH * (c + 1)])
        cs_sb[c] = t_cs
        t_sn = consts.tile([128, CH], F32, tag=f"sn{c}", name="t_sn")
        nc.sync.dma_start(t_sn, sn[:, CH * c:CH * (c + 1)])
        sn_sb[c] = t_sn

    def _load_chunk(c):
        t_x = consts.tile([128, 8 * CH], BF16, tag=f"xt{c}", name="t_x")
        nc.sync.dma_start(t_x, xt[:, 8 * CH * c:8 * CH * (c + 1)])
        xt_sb[c] = [(t_x, 0)]
        _load_cssn(c)

    def xt_d(c, d, lo=0, hi=CH):
        for t_x, dbase in reversed(xt_sb[c]):
            if d >= dbase:
                o = CH * (d - dbase)
                return t_x[:, o + lo:o + hi]

    xt_sb[0] = xt0h
    _load_cssn(0)
    # lower-triangular keep-mask (keep iff q_local >= kv_local)
    tri = consts.tile([128, 128], BF16)
    nc.vector.memset(tri, 1.0)
    nc.gpsimd.affine_select(tri, tri, pattern=[[1, 128]],
                            compare_op=mybir.AluOpType.is_ge, fill=0.0,
                            base=0, channel_multiplier=-1)

    # persistent activations: head-contiguous RoPE'd Q/K in bf16
    q2a = persist.tile([128, L], BF16)   # heads 0,1
    q2b = persist.tile([128, L], BF16)   # heads 2,3
    k2a = persist.tile([128, L], BF16)
    k2b = persist.tile([128, L], BF16)
    v_sb = []
    for t in range(NT):
        t_v = persist.tile([128, HPC * 65], BF16, tag=f"v{t}")
        v_sb.append(t_v)
        nc.vector.memset(t_v[:, 64:HPC * 65:65], 1.0)
    ho = []
    for j in range(2):
        t_ho = persist.tile([128, L], F32R, tag=f"ho{j}")
        ho.append(t_ho)

    wv_sb = []
    wo_sb = []

    def _load_wv():
        wvall = consts.tile([128, 8 * 256], BF16)
        nc.sync.dma_start(wvall, wv[:, :])
        return [wvall[:, 256 * d:256 * (d + 1)] for d in range(8)]

    def _load_wo():
        res = []
        for j in range(2):
            t_wo = consts.tile([128, DM], F32R, tag=f"wo{j}")
            nc.sync.dma_start(t_wo, wo[128 * j:128 * j + 128, :].bitcast(F32R))
            res.append(t_wo)
        return res

    def rope_muls(psrc, cs_c, sn_c):
        """cos/sin products of a [128, CH] PSUM pair-tile (releases psrc)."""
        tmpc = ropet.tile([128, CH], BF16, tag="tc")
        tmps = ropet.tile([128, CH], BF16, tag="ts")
        nc.vector.tensor_mul(tmpc, psrc, cs_c)
        nc.vector.tensor_mul(tmps, psrc, sn_c)
        return tmpc, tmps

    def rope_combine(tmpc, tmps, dst, lsl):
        tmpw = ropet.tile([128, CH], BF16, tag="tw")
        nc.vector.stream_shuffle(tmpw, tmps, mask=_SWAP16)
        # combine on the (otherwise idle) Pool engine: all-SBUF bf16
        nc.gpsimd.tensor_add(dst[:, lsl], tmpc, tmpw)

    def emit_proj_q(c):
        """Q projection matmuls + cos/sin products for chunk c (s0 tag)."""
        psq = ps.tile([128, 2 * CH], F32, tag="scP0")
        for d in range(8):
            nc.tensor.matmul(psq[:, 0:CH], wq_sl(d, 0, 128),
                             xt_d(c, d), start=(d == 0), stop=(d == 7))
            nc.tensor.matmul(psq[:, CH:2 * CH], wq_sl(d, 128, 256),
                             xt_d(c, d), start=(d == 0), stop=(d == 7))
        ca = rope_muls(psq[:, 0:CH], cs_sb[c], sn_sb[c])
        cb = rope_muls(psq[:, CH:2 * CH], cs_sb[c], sn_sb[c])
        return ca, cb

    def emit_proj_k_mms(c):
        psk = ps.tile([128, 2 * CH], F32, tag="scP1")
        for d in range(8):
            nc.tensor.matmul(psk[:, 0:CH], wq_sl(d, 256, 384),
                             xt_d(c, d), start=(d == 0), stop=(d == 7))
            nc.tensor.matmul(psk[:, CH:2 * CH], wq_sl(d, 384, 512),
                             xt_d(c, d), start=(d == 0), stop=(d == 7))
        return psk

    def emit_proj_v_mms(c, tags=None):
        # V: 4 l-tiles of [128, 256]; on the s0 tag (after psq) by default,
        # or on two given (untouched) av tags during startup
        if tags is None:
            psv = ps.tile([128, 2 * CH], F32, tag="scP0")
            tiles = [(psv, 0), (psv, 512)]
        else:
            tiles = []
            for tg in tags:
                ps_vh = ps.tile([128, CH], F32, tag=tg, name="ps_vh")
                tiles.append((ps_vh, 0))
        for i in range(4):
            lo = 128 * i
            tile_, base = tiles[i // 2]
            col = base + 256 * (i % 2)
            for d in range(8):
                nc.tensor.matmul(tile_[:, col:col + 256],
                                 xt_d(c, d, lo, lo + 128),
                                 wv_sb[d], start=(d == 0), stop=(d == 7))
        return tiles

    def emit_ropek_muls(c, psk):
        ca = rope_muls(psk[:, 0:CH], cs_sb[c], sn_sb[c])
        cb = rope_muls(psk[:, CH:2 * CH], cs_sb[c], sn_sb[c])
        return ca, cb

    def emit_v_copies(c, tiles):
        for i in range(4):
            t = 4 * c + i
            tile_, base = tiles[i // 2]
            col = base + 256 * (i % 2)
            vdst = v_sb[t][:].rearrange("p (h x) -> p h x", x=65)[:, :, 0:64]
            vsrc = tile_[:, col:col + 256].rearrange(
                "p (h x) -> p h x", x=64)
            nc.scalar.copy(vdst, vsrc)

    def emit_rope_combines(c, qc, kc):
        lsl = slice(CH * c, CH * (c + 1))
        rope_combine(*qc[0], q2a, lsl)
        rope_combine(*qc[1], q2b, lsl)
        rope_combine(*kc[0], k2a, lsl)
        rope_combine(*kc[1], k2b, lsl)

    def emit_scores_tile(c, t):
        """Scores + exp (+ causal mask) for kv tile t of chunk c.
        Returns the two bf16 expt tiles and the causal column offset."""
        ksl = slice(128 * t, 128 * t + 128)
        diag = (t // 4 == c)
        off = 128 * t - CH * c if diag else 0
        strip0 = ps.tile([128, 2 * CH], F32, tag="scP0")
        strip1 = ps.tile([128, 2 * CH], F32, tag="scP1")
        strips = [strip0, strip1]
        for h in range(HPC):
            k2 = (k2a, k2b)[h // 2]
            q2 = (q2a, q2b)[h // 2]
            hsl = slice(64 * (h % 2), 64 * (h % 2) + 64)
            pss = strips[h // 2][:, CH * (h % 2) + off:CH * (h % 2 + 1)]
            nc.tensor.matmul(pss, k2[hsl, ksl],
                             q2[hsl, CH * c + off:CH * (c + 1)],
                             start=True, stop=True,
                             tile_position=(64 * (h % 2), 0))
        expts = []
        for p in range(2):
            strip = strips[p]
            expt = epool.tile([128, 2 * CH], BF16, tag="expt")
            esrc = strip[:].rearrange("q (h x) -> q h x", x=CH)[:, :, off:]
            edst = expt[:].rearrange("q (h x) -> q h x", x=CH)[:, :, off:]
            nc.scalar.activation(edst, esrc, AF.Exp, scale=0.125)
            if diag:
                # triangular mask on the 128-wide diagonal block
                for hh in range(2):
                    blk = slice(CH * hh + off, CH * hh + off + 128)
                    nc.vector.tensor_mul(expt[:, blk], expt[:, blk], tri)
            expts.append(expt)
        return expts, off

    def emit_av_tile(c, t, av, expts, off):
        ntile = 4 * c + 4
        for p in range(2):
            for hh in range(2):
                h = 2 * p + hh
                nc.tensor.matmul(av[h][:, off:],
                                 v_sb[t][:, 65 * h:65 * h + 65],
                                 expts[p][:, CH * hh + off:CH * (hh + 1)],
                                 start=(t == 0), stop=(t == ntile - 1))

    def alloc_av():
        av = []
        for h in range(HPC):
            t_av = ps.tile([65, CH], F32, tag=f"av{h}", name="t_av")
            av.append(t_av)
        return av

    def emit_attention(c, av, t0=0, pre=()):
        """AV for prefetched tiles `pre`, then full tiles t0..ntile-1."""
        for t, (expts, off) in enumerate(pre):
            emit_av_tile(c, t, av, expts, off)
        for t in range(t0, 4 * c + 4):
            expts, off = emit_scores_tile(c, t)
            emit_av_tile(c, t, av, expts, off)

    def emit_recips(c, av):
        """Reciprocal (DVE) + partition broadcast (Pool) of softmax denoms."""
        bcs = []
        for h in range(HPC):
            inv = ipool.tile([1, CH], F32R, tag=f"inv{h}")
            with nc.allow_low_precision(reason="tf32 softmax denom"):
                nc.vector.reciprocal(inv, av[h][64:65, :])
            bch = ipool.tile([64, CH], F32R, tag=f"bc{h}")
            nc.gpsimd.partition_broadcast(bch, inv)
            bcs.append(bch)
        return bcs

    def emit_norm_pair(c, av, p):
        qsl = slice(CH * c, CH * (c + 1))
        bcs = []
        for hh in range(2):
            h = 2 * p + hh
            inv = ipool.tile([1, CH], F32R, tag=f"inv{h}")
            with nc.allow_low_precision(reason="tf32 softmax denom"):
                nc.vector.reciprocal(inv, av[h][64:65, :])
            bch = ipool.tile([64, CH], F32R, tag=f"bc{h}")
            nc.gpsimd.partition_broadcast(bch, inv)
            bcs.append(bch)
        for hh in range(2):
            h = 2 * p + hh
            nc.vector.tensor_mul(
                ho[h // 2][64 * (h % 2):64 * (h % 2) + 64, qsl],
                av[h][0:64, :], bcs[hh])

    def emit_homuls(c, av, bcs):
        qsl = slice(CH * c, CH * (c + 1))
        for h in range(HPC):
            nc.vector.tensor_mul(
                ho[h // 2][64 * (h % 2):64 * (h % 2) + 64, qsl],
                av[h][0:64, :], bcs[h])

    def emit_outproj_group(c, g):
        """Half of chunk c's output projection (2 l-tiles x 2 column halves)
        on av0..av3 — each tag used once per group, j rounds interleaved,
        staging copies alternating DVE/ACT into one bf16 row-tile per lt."""
        tiles = []
        for i, lt in enumerate((4 * c + 2 * g, 4 * c + 2 * g + 1)):
            for oc in range(2):
                ps_o = ps.tile([128, 512], F32, tag=f"av{2 * i + oc}",
                               name="ps_o")
                tiles.append((ps_o, lt, oc))
        for j in range(2):
            for ps_o, lt, oc in tiles:
                osl = slice(512 * oc, 512 * oc + 512)
                nc.tensor.matmul(ps_o, ho[j][:, 128 * lt:128 * lt + 128],
                                 wo_sb[j][:, osl],
                                 start=(j == 0), stop=(j == 1))
        for i, lt in enumerate((4 * c + 2 * g, 4 * c + 2 * g + 1)):
            o_sb = opool.tile([128, DM], BF16, tag="o")
            nc.vector.tensor_copy(o_sb[:, 0:512], tiles[2 * i][0][:])
            nc.scalar.copy(o_sb[:, 512:DM], tiles[2 * i + 1][0][:])
            nc.sync.dma_start(out[128 * lt:128 * lt + 128, :], o_sb)

    # ---- software pipeline ----
    wv_sb.extend(_load_wv())
    _load_chunk(1)
    # startup: V projections go on untouched av tags (no WAR on the rope
    # muls); chunk-0 scores are prefetched with their AV matmuls deferred
    # until the V copies land
    qc0 = emit_proj_q(0)
    psk0 = emit_proj_k_mms(0)
    kc0 = emit_ropek_muls(0, psk0)
    emit_rope_combines(0, qc0, kc0)
    psv0 = emit_proj_v_mms(0, tags=("av0", "av1"))
    _load_chunk(2)
    wo_sb.extend(_load_wo())
    qc1 = emit_proj_q(1)
    psk1 = emit_proj_k_mms(1)
    kc1 = emit_ropek_muls(1, psk1)
    psv1 = emit_proj_v_mms(1, tags=("av2", "av3"))
    pre = [emit_scores_tile(0, t) for t in range(4)]
    emit_v_copies(0, psv0)
    emit_v_copies(1, psv1)
    emit_rope_combines(1, qc1, kc1)
    for c in range(2):
        av = alloc_av()
        emit_attention(c, av, t0=len(pre), pre=pre)
        # engine-queue order: PE att|Q|K|V|outproj|pre-scores; DVE ropeQ
        # muls|recips|homuls|ropeK muls|v-copies|o-copies|shuffles — the
        # normalize chain hides under the projection matmuls
        qc = emit_proj_q(c + 2)
        psk = emit_proj_k_mms(c + 2)
        bcs = emit_recips(c, av)
        psv = emit_proj_v_mms(c + 2)
        emit_homuls(c, av, bcs)
        kc = emit_ropek_muls(c + 2, psk)
        emit_v_copies(c + 2, psv)
        if c + 3 < NCH:
            _load_chunk(c + 3)
        emit_outproj_group(c, 0)
        emit_outproj_group(c, 1)
        pre = [emit_scores_tile(c + 1, t) for t in range(6)]
        emit_rope_combines(c + 2, qc, kc)
    # c = 2: no projection filler left — prefetch the first scores of
    # chunk 3 (deferring their AV matmuls) to keep PE fed during norm(2)
    av = alloc_av()
    emit_attention(2, av, t0=len(pre), pre=pre)
    bcs = emit_recips(2, av)
    pre = [emit_scores_tile(3, t) for t in range(8)]
    emit_homuls(2, av, bcs)
    emit_outproj_group(2, 0)
    emit_outproj_group(2, 1)
    # c = 3: pair-interleaved normalize so outproj round 0 starts early
    av = alloc_av()
    emit_attention(3, av, t0=len(pre), pre=pre)
    emit_norm_pair(3, av, 0)
    emit_norm_pair(3, av, 1)
    emit_outproj_group(3, 0)
    emit_outproj_group(3, 1)


def _build_nc():
    nc = bacc.Bacc("TRN2", target_bir_lowering=False, debug=False,
                   enable_asserts=False, num_devices=8)
    ins = {
        "xt": nc.dram_tensor("xt", [128, NCH * 8 * CH], BF16,
                             kind="ExternalInput").ap(),
        "wq": nc.dram_tensor("wq", [128, 8 * 512], BF16,
                             kind="ExternalInput").ap(),
        "wv": nc.dram_tensor("wv", [128, 8 * 256], BF16,
                             kind="ExternalInput").ap(),
        "wo": nc.dram_tensor("wo", [256, DM], F32, kind="ExternalInput").ap(),
        "cs": nc.dram_tensor("cs", [128, L], F32, kind="ExternalInput").ap(),
        "sn": nc.dram_tensor("sn", [128, L], F32, kind="ExternalInput").ap(),
    }
    outs = {"out": nc.dram_tensor("out", [L, DM], BF16, kind="ExternalOutput").ap()}
    with tile.TileContext(nc) as tc:
        _attn_kernel(tc, outs, ins)
    nc.compile()
    return nc


def _host_shard(X, token_positions, Wqkv, Wout):
    """Build the 8 per-core input maps."""
    X = np.asarray(X, dtype=np.float32)
    Wqkv = np.asarray(Wqkv, dtype=np.float32)
    Wout = np.asarray(Wout, dtype=np.float32)
    pos = np.asarray(token_positions)

    # Per-head dim order: [e0..e15, o0..o15, e16..e31, o16..o31] so the RoPE
    # partner (even<->odd of the same freq) is p^16 within a 32-quadrant.
    def head_order(base):
        return ([base + 2 * k for k in range(16)] +
                [base + 2 * k + 1 for k in range(16)] +
                [base + 2 * k for k in range(16, 32)] +
                [base + 2 * k + 1 for k in range(16, 32)])

    # RoPE tables in float32 arithmetic to mirror the f32 reference
    k = np.arange(DK // 2, dtype=np.float32)
    inv_freq = (np.float32(1.0) /
                np.power(np.float32(THETA), (np.float32(2.0) * k) / np.float32(DK)))
    inv_freq = inv_freq.astype(np.float32)
    # freq index per partition within a 64-block: [0:16]=f0..15, [16:32]=f0..15,
    # [32:48]=f16..31, [48:64]=f16..31; sign +1 on top blocks, -1 on bot blocks
    fidx = np.concatenate([np.arange(16), np.arange(16),
                           np.arange(16, 32), np.arange(16, 32)])
    sgn = np.concatenate([np.ones(16), -np.ones(16),
                          np.ones(16), -np.ones(16)]).astype(np.float32)
    fidx = np.tile(fidx, 2)           # 128 partitions (2 heads per tile)
    sgn = np.tile(sgn, 2)
    ang = (pos.astype(np.float32)[:, None, :] *
           inv_freq[fidx][None, :, None]).astype(np.float32)   # [B, 128, L]
    cs_all = np.cos(ang).astype(np.float32)
    sn_all = (np.sin(ang) * sgn[None, :, None]).astype(np.float32)

    in_maps = []
    for core in range(8):
        b, g = divmod(core, HPC)
        heads = [HPC * g + hh for hh in range(HPC)]
        q_rows, k_rows = [], []
        for h in heads:
            q_rows += head_order(DK * h)
            k_rows += head_order(DM + DK * h)
        # tile-major packs: [128, d-blocks * cols] so each group is one
        # contiguous DMA on the device side
        wq_c = (Wqkv[q_rows + k_rows, :].T.astype(ml_dtypes.bfloat16)
                .reshape(8, 128, 512).transpose(1, 0, 2).reshape(128, -1))
        v_rows = [2 * DM + DK * h + j for h in heads for j in range(DK)]
        wv_c = (Wqkv[v_rows, :].T.astype(ml_dtypes.bfloat16)
                .reshape(8, 128, 256).transpose(1, 0, 2).reshape(128, -1))
        wo_c = np.ascontiguousarray(Wout[:, 256 * g:256 * (g + 1)].T)
        xt_c = (X[b].T.astype(ml_dtypes.bfloat16)
                .reshape(8, 128, NCH, CH).transpose(1, 2, 0, 3)
                .reshape(128, -1))
        in_maps.append({
            "xt": np.ascontiguousarray(xt_c),
            "wq": np.ascontiguousarray(wq_c),
            "wv": np.ascontiguousarray(wv_c),
            "wo": wo_c,
            "cs": np.ascontiguousarray(cs_all[b]),
            "sn": np.ascontiguousarray(sn_all[b]),
        })
    return in_maps


def kernel(X, token_positions, Wqkv, Wout, _trace=False):
    if "nc" not in _cache:
        _cache["nc"] = _build_nc()
    nc = _cache["nc"]
    in_maps = _host_shard(X, token_positions, Wqkv, Wout)
    res = run_bass_kernel_spmd(nc, in_maps, list(range(8)), trace=_trace)
    _cache["last_results"] = res
    out = np.zeros((B, L, DM), dtype=np.float32)
    for core in range(8):
        out[core // HPC] += np.asarray(res.results[core]["out"],
                                       dtype=np.float32)
    return out


# revision 75
# speedup vs baseline: 1.0028x; 1.0028x over previous
"""Causal multi-head attention with RoPE on 8 Trainium2 NeuronCores.

Problem: B=2, L=2048, D_MODEL=1024, N_HEADS=16, D_K=64, theta=10000.
Sharding: data parallel on batch (2) x tensor parallel on heads (4 groups of
4 heads) = 8 cores. Each core computes its 4 heads' attention plus a partial
output projection; partials are summed on the host (Megatron row-parallel).

Per-core device design (v4):
- Q/K live head-contiguous in bf16: each head owns 64 partitions, with its
  RoPE top (even) / bot (odd) dims interleaved in 16-blocks so the rotation
  partner is always p^16 within a 32-quadrant.  RoPE is then 2 full-width
  DVE muls (cos / sign-folded sin) + one stream_shuffle(+-16) + one bf16
  Pool-engine add.  Scores become a SINGLE K=64 matmul per head-tile (the
  cost model charges a matmul by its moving size regardless of K, so a K=32
  top/bot pair would pay 2x), with causal column-slicing on diagonal tiles.
- PSUM: s0/s1 are 2-bank strips (2 heads of scoresT each), a0..a3 1-bank AV
  accumulators (an appended ones-column on V gives the softmax denominator).
  Projections use only s0/s1 (QA+QB / KA+KB / V0..V3 share single 1024-wide
  instances; the two startup V projections borrow the still-untouched av
  tags), so attention's AV tags never WAR-block on the projection pipeline.
- Normalization is PE-free: reciprocal (DVE) -> partition_broadcast (Pool)
  -> scale-copy into ho (DVE).  Inputs arrive in tile-major host layouts so
  each logical group is one contiguous DMA (HWDGE costs a fixed ~625ns per
  instruction); the output is written bf16 and upcast host-side.
- Per chunk c the emission interleaves engines so their queues stay in
  need-order: att(c) | projQ(c+2) mms+muls | recips(c) | projK mms |
  projV mms | homuls(c) | ropeK muls | v-copies | outproj(c) |
  pre-scores(c+1) (AV deferred) | rope combines(c+2).  The prefetched
  score tiles keep PE fed where attention is exp(ACT)-bound.
"""
import numpy as np
import ml_dtypes
from contextlib import ExitStack

import concourse.bacc as bacc
import concourse.bass as bass
import concourse.mybir as mybir
import concourse.tile as tile
from concourse._compat import with_exitstack
from concourse.bass_utils import run_bass_kernel_spmd

F32 = mybir.dt.float32
F32R = mybir.dt.float32r
BF16 = mybir.dt.bfloat16

B, L, DM, NH, DK = 2, 2048, 1024, 16, 64
HPC = 4              # heads per core
THETA = 10000.0
CH = 512             # q/l chunk
NT = L // 128        # 16 kv tiles
NCH = L // CH        # 4 chunks

_cache = {}

# stream_shuffle mask: swap 16-blocks within each 32-quadrant (p <-> p^16)
_SWAP16 = list(range(16, 32)) + list(range(16))


@with_exitstack
def _attn_kernel(ctx: ExitStack, tc: tile.TileContext, outs, ins):
    nc = tc.nc
    xt, wq, wv, wo = ins["xt"], ins["wq"], ins["wv"], ins["wo"]
    cs, sn = ins["cs"], ins["sn"]
    out = outs["out"]
    AF = mybir.ActivationFunctionType

    consts = ctx.enter_context(tc.tile_pool(name="consts", bufs=1))
    persist = ctx.enter_context(tc.tile_pool(name="persist", bufs=1))
    ps = ctx.enter_context(tc.tile_pool(name="ps", bufs=1, space="PSUM"))
    ropet = ctx.enter_context(tc.tile_pool(name="ropet", bufs=6))
    epool = ctx.enter_context(tc.tile_pool(name="epool", bufs=14))
    ipool = ctx.enter_context(tc.tile_pool(name="ipool", bufs=2))
    opool = ctx.enter_context(tc.tile_pool(name="opool", bufs=4))

    # ---- weights / inputs ----
    # xt is loaded as full [128, L] rows: one descriptor per partition for
    # the whole row keeps the (descriptor-bound) HWDGE cost per byte low.
    # wq and xt are interleaved per-d so the d-chain starts early.
    # All inputs arrive in tile-major host layouts so each logical group is
    # ONE contiguous DMA (HWDGE cost is a fixed ~625ns per instruction).
    wqh = []
    xt0h = []
    for piece in range(4):
        t_wqh = consts.tile([128, 2 * 512], BF16, tag=f"wq{piece}")
        nc.sync.dma_start(t_wqh, wq[:, 1024 * piece:1024 * (piece + 1)])
        wqh.append(t_wqh)
        t_x0 = consts.tile([128, 2 * CH], BF16, tag=f"xt0_{piece}",
                           name="t_x0")
        nc.sync.dma_start(t_x0, xt[:, 2 * CH * piece:2 * CH * (piece + 1)])
        xt0h.append((t_x0, 2 * piece))

    def wq_sl(d, lo, hi):
        return wqh[d // 2][:, 512 * (d % 2) + lo:512 * (d % 2) + hi]

    xt_sb = [None] * NCH    # [chunk] -> list of (tile, d-offset) halves
    cs_sb = [None] * NCH
    sn_sb = [None] * NCH

    def _load_cssn(c):
        t_cs = consts.tile([128, CH], F32, tag=f"cs{c}", name="t_cs")
        nc.sync.dma_start(t_cs, cs[:, CH * c:CH * (c + 1)])
        cs_sb[c] = t_cs
        t_sn = consts.tile([128, CH], F32, tag=f"sn{c}", name="t_sn")
        nc.sync.dma_start(t_sn, sn[:, CH * c:CH * (c + 1)])
        sn_sb[c] = t_sn

    def _load_chunk(c):
        t_x = consts.tile([128, 8 * CH], BF16, tag=f"xt{c}", name="t_x")
        nc.sync.dma_start(t_x, xt[:, 8 * CH * c:8 * CH * (c + 1)])
        xt_sb[c] = [(t_x, 0)]
        _load_cssn(c)

    def xt_d(c, d, lo=0, hi=CH):
        for t_x, dbase in reversed(xt_sb[c]):
            if d >= dbase:
                o = CH * (d - dbase)
                return t_x[:, o + lo:o + hi]

    xt_sb[0] = xt0h
    _load_cssn(0)
    # lower-triangular keep-mask (keep iff q_local >= kv_local)
    tri = consts.tile([128, 128], BF16)
    nc.vector.memset(tri, 1.0)
    nc.gpsimd.affine_select(tri, tri, pattern=[[1, 128]],
                            compare_op=mybir.AluOpType.is_ge, fill=0.0,
                            base=0, channel_multiplier=-1)

    # persistent activations: head-contiguous RoPE'd Q/K in bf16
    q2a = persist.tile([128, L], BF16)   # heads 0,1
    q2b = persist.tile([128, L], BF16)   # heads 2,3
    k2a = persist.tile([128, L], BF16)
    k2b = persist.tile([128, L], BF16)
    v_sb = []
    for t in range(NT):
        t_v = persist.tile([128, HPC * 65], BF16, tag=f"v{t}")
        v_sb.append(t_v)
        nc.vector.memset(t_v[:, 64:HPC * 65:65], 1.0)
    ho = []
    for j in range(2):
        t_ho = persist.tile([128, L], F32R, tag=f"ho{j}")
        ho.append(t_ho)

    wv_sb = []
    wo_sb = []

    def _load_wv():
        wvall = consts.tile([128, 8 * 256], BF16)
        nc.sync.dma_start(wvall, wv[:, :])
        return [wvall[:, 256 * d:256 * (d + 1)] for d in range(8)]

    def _load_wo():
        res = []
        for j in range(2):
            t_wo = consts.tile([128, DM], F32R, tag=f"wo{j}")
            nc.sync.dma_start(t_wo, wo[128 * j:128 * j + 128, :].bitcast(F32R))
            res.append(t_wo)
        return res

    def rope_muls(psrc, cs_c, sn_c):
        """cos/sin products of a [128, CH] PSUM pair-tile (releases psrc)."""
        tmpc = ropet.tile([128, CH], BF16, tag="tc")
        tmps = ropet.tile([128, CH], BF16, tag="ts")
        nc.vector.tensor_mul(tmpc, psrc, cs_c)
        nc.vector.tensor_mul(tmps, psrc, sn_c)
        return tmpc, tmps

    def rope_combine(tmpc, tmps, dst, lsl):
        tmpw = ropet.tile([128, CH], BF16, tag="tw")
        nc.vector.stream_shuffle(tmpw, tmps, mask=_SWAP16)
        # combine on the (otherwise idle) Pool engine: all-SBUF bf16
        nc.gpsimd.tensor_add(dst[:, lsl], tmpc, tmpw)

    def emit_proj_q(c):
        """Q projection matmuls + cos/sin products for chunk c (s0 tag)."""
        psq = ps.tile([128, 2 * CH], F32, tag="scP0")
        for d in range(8):
            nc.tensor.matmul(psq[:, 0:CH], wq_sl(d, 0, 128),
                             xt_d(c, d), start=(d == 0), stop=(d == 7))
            nc.tensor.matmul(psq[:, CH:2 * CH], wq_sl(d, 128, 256),
                             xt_d(c, d), start=(d == 0), stop=(d == 7))
        ca = rope_muls(psq[:, 0:CH], cs_sb[c], sn_sb[c])
        cb = rope_muls(psq[:, CH:2 * CH], cs_sb[c], sn_sb[c])
        return ca, cb

    def emit_proj_k_mms(c):
        psk = ps.tile([128, 2 * CH], F32, tag="scP1")
        for d in range(8):
            nc.tensor.matmul(psk[:, 0:CH], wq_sl(d, 256, 384),
                             xt_d(c, d), start=(d == 0), stop=(d == 7))
            nc.tensor.matmul(psk[:, CH:2 * CH], wq_sl(d, 384, 512),
                             xt_d(c, d), start=(d == 0), stop=(d == 7))
        return psk

    def emit_proj_v_mms(c, tags=None):
        # V: 4 l-tiles of [128, 256]; on the s0 tag (after psq) by default,
        # or on two given (untouched) av tags during startup
        if tags is None:
            psv = ps.tile([128, 2 * CH], F32, tag="scP0")
            tiles = [(psv, 0), (psv, 512)]
        else:
            tiles = []
            for tg in tags:
                ps_vh = ps.tile([128, CH], F32, tag=tg, name="ps_vh")
                tiles.append((ps_vh, 0))
        for i in range(4):
            lo = 128 * i
            tile_, base = tiles[i // 2]
            col = base + 256 * (i % 2)
            for d in range(8):
                nc.tensor.matmul(tile_[:, col:col + 256],
                                 xt_d(c, d, lo, lo + 128),
                                 wv_sb[d], start=(d == 0), stop=(d == 7))
        return tiles

    def emit_ropek_muls(c, psk):
        ca = rope_muls(psk[:, 0:CH], cs_sb[c], sn_sb[c])
        cb = rope_muls(psk[:, CH:2 * CH], cs_sb[c], sn_sb[c])
        return ca, cb

    def emit_v_copies(c, tiles):
        for i in range(4):
            t = 4 * c + i
            tile_, base = tiles[i // 2]
            col = base + 256 * (i % 2)
            vdst = v_sb[t][:].rearrange("p (h x) -> p h x", x=65)[:, :, 0:64]
            vsrc = tile_[:, col:col + 256].rearrange(
                "p (h x) -> p h x", x=64)
            nc.scalar.copy(vdst, vsrc)

    def emit_rope_combines(c, qc, kc):
        lsl = slice(CH * c, CH * (c + 1))
        rope_combine(*qc[0], q2a, lsl)
        rope_combine(*qc[1], q2b, lsl)
        rope_combine(*kc[0], k2a, lsl)
        rope_combine(*kc[1], k2b, lsl)

    def emit_scores_tile(c, t):
        """Scores + exp (+ causal mask) for kv tile t of chunk c.
        Returns the two bf16 expt tiles and the causal column offset."""
        ksl = slice(128 * t, 128 * t + 128)
        diag = (t // 4 == c)
        off = 128 * t - CH * c if diag else 0
        strip0 = ps.tile([128, 2 * CH], F32, tag="scP0")
        strip1 = ps.tile([128, 2 * CH], F32, tag="scP1")
        strips = [strip0, strip1]
        for h in range(HPC):
            k2 = (k2a, k2b)[h // 2]
            q2 = (q2a, q2b)[h // 2]
            hsl = slice(64 * (h % 2), 64 * (h % 2) + 64)
            pss = strips[h // 2][:, CH * (h % 2) + off:CH * (h % 2 + 1)]
            nc.tensor.matmul(pss, k2[hsl, ksl],
                             q2[hsl, CH * c + off:CH * (c + 1)],
                             start=True, stop=True,
                             tile_position=(64 * (h % 2), 0))
        expts = []
        for p in range(2):
            strip = strips[p]
            expt = epool.tile([128, 2 * CH], BF16, tag="expt")
            esrc = strip[:].rearrange("q (h x) -> q h x", x=CH)[:, :, off:]
            edst = expt[:].rearrange("q (h x) -> q h x", x=CH)[:, :, off:]
            nc.scalar.activation(edst, esrc, AF.Exp, scale=0.125)
            if diag:
                # triangular mask on the 128-wide diagonal block
                for hh in range(2):
                    blk = slice(CH * hh + off, CH * hh + off + 128)
                    nc.vector.tensor_mul(expt[:, blk], expt[:, blk], tri)
            expts.append(expt)
        return expts, off

    def emit_av_tile(c, t, av, expts, off):
        ntile = 4 * c + 4
        for p in range(2):
            for hh in range(2):
                h = 2 * p + hh
                nc.tensor.matmul(av[h][:, off:],
                                 v_sb[t][:, 65 * h:65 * h + 65],
                                 expts[p][:, CH * hh + off:CH * (hh + 1)],
                                 start=(t == 0), stop=(t == ntile - 1))

    def alloc_av():
        av = []
        for h in range(HPC):
            t_av = ps.tile([65, CH], F32, tag=f"av{h}", name="t_av")
            av.append(t_av)
        return av

    def emit_attention(c, av, t0=0, pre=()):
        """AV for prefetched tiles `pre`, then full tiles t0..ntile-1."""
        for t, (expts, off) in enumerate(pre):
            emit_av_tile(c, t, av, expts, off)
        for t in range(t0, 4 * c + 4):
            expts, off = emit_scores_tile(c, t)
            emit_av_tile(c, t, av, expts, off)

    def emit_recips(c, av):
        """Reciprocal (DVE) + partition broadcast (Pool) of softmax denoms."""
        bcs = []
        for h in range(HPC):
            inv = ipool.tile([1, CH], F32R, tag=f"inv{h}")
            with nc.allow_low_precision(reason="tf32 softmax denom"):
                nc.vector.reciprocal(inv, av[h][64:65, :])
            bch = ipool.tile([64, CH], F32R, tag=f"bc{h}")
            nc.gpsimd.partition_broadcast(bch, inv)
            bcs.append(bch)
        return bcs

    def emit_norm_pair(c, av, p):
        qsl = slice(CH * c, CH * (c + 1))
        bcs = []
        for hh in range(2):
            h = 2 * p + hh
            inv = ipool.tile([1, CH], F32R, tag=f"inv{h}")
            with nc.allow_low_precision(reason="tf32 softmax denom"):
                nc.vector.reciprocal(inv, av[h][64:65, :])
            bch = ipool.tile([64, CH], F32R, tag=f"bc{h}")
            nc.gpsimd.partition_broadcast(bch, inv)
            bcs.append(bch)
        for hh in range(2):
            h = 2 * p + hh
            nc.vector.tensor_mul(
                ho[h // 2][64 * (h % 2):64 * (h % 2) + 64, qsl],
                av[h][0:64, :], bcs[hh])

    def emit_homuls(c, av, bcs):
        qsl = slice(CH * c, CH * (c + 1))
        for h in range(HPC):
            nc.vector.tensor_mul(
                ho[h // 2][64 * (h % 2):64 * (h % 2) + 64, qsl],
                av[h][0:64, :], bcs[h])

    def emit_outproj_group(c, g):
        """Half of chunk c's output projection (2 l-tiles x 2 column halves)
        on av0..av3 — each tag used once per group, j rounds interleaved,
        staging copies alternating DVE/ACT into one bf16 row-tile per lt."""
        tiles = []
        for i, lt in enumerate((4 * c + 2 * g, 4 * c + 2 * g + 1)):
            for oc in range(2):
                ps_o = ps.tile([128, 512], F32, tag=f"av{2 * i + oc}",
                               name="ps_o")
                tiles.append((ps_o, lt, oc))
        for j in range(2):
            for ps_o, lt, oc in tiles:
                osl = slice(512 * oc, 512 * oc + 512)
                nc.tensor.matmul(ps_o, ho[j][:, 128 * lt:128 * lt + 128],
                                 wo_sb[j][:, osl],
                                 start=(j == 0), stop=(j == 1))
        for i, lt in enumerate((4 * c + 2 * g, 4 * c + 2 * g + 1)):
            o_sb = opool.tile([128, DM], BF16, tag="o")
            nc.vector.tensor_copy(o_sb[:, 0:512], tiles[2 * i][0][:])
            nc.scalar.copy(o_sb[:, 512:DM], tiles[2 * i + 1][0][:])
            nc.sync.dma_start(out[128 * lt:128 * lt + 128, :], o_sb)

    # ---- software pipeline ----
    wv_sb.extend(_load_wv())
    _load_chunk(1)
    # startup: V projections go on untouched av tags (no WAR on the rope
    # muls); chunk-0 scores are prefetched with their AV matmuls deferred
    # until the V copies land
    qc0 = emit_proj_q(0)
    psk0 = emit_proj_k_mms(0)
    kc0 = emit_ropek_muls(0, psk0)
    emit_rope_combines(0, qc0, kc0)
    psv0 = emit_proj_v_mms(0, tags=("av0", "av1"))
    _load_chunk(2)
    wo_sb.extend(_load_wo())
    qc1 = emit_proj_q(1)
    psk1 = emit_proj_k_mms(1)
    kc1 = emit_ropek_muls(1, psk1)
    psv1 = emit_proj_v_mms(1, tags=("av2", "av3"))
    pre = [emit_scores_tile(0, t) for t in range(4)]
    emit_v_copies(0, psv0)
    emit_v_copies(1, psv1)
    emit_rope_combines(1, qc1, kc1)
    for c in range(2):
        av = alloc_av()
        emit_attention(c, av, t0=len(pre), pre=pre)
        # engine-queue order: PE att|Q|K|V|outproj|pre-scores; DVE ropeQ
        # muls|recips|homuls|ropeK muls|v-copies|o-copies|shuffles — the
        # normalize chain hides under the projection matmuls
        qc = emit_proj_q(c + 2)
        psk = emit_proj_k_mms(c + 2)
        bcs = emit_recips(c, av)
        psv = emit_proj_v_mms(c + 2)
        emit_homuls(c, av, bcs)
        kc = emit_ropek_muls(c + 2, psk)
        emit_v_copies(c + 2, psv)
        if c + 3 < NCH:
            _load_chunk(c + 3)
        emit_outproj_group(c, 0)
        emit_outproj_group(c, 1)
        pre = [emit_scores_tile(c + 1, t) for t in range(6)]
        emit_rope_combines(c + 2, qc, kc)
    # c = 2: no projection filler left — prefetch the first scores of
    # chunk 3 (deferring their AV matmuls) to keep PE fed during norm(2)
    av = alloc_av()
    emit_attention(2, av, t0=len(pre), pre=pre)
    bcs = emit_recips(2, av)
    pre = [emit_scores_tile(3, t) for t in range(6)]
    emit_homuls(2, av, bcs)
    emit_outproj_group(2, 0)
    emit_outproj_group(2, 1)
    # c = 3: pair-interleaved normalize so outproj round 0 starts early
    av = alloc_av()
    emit_attention(3, av, t0=len(pre), pre=pre)
    emit_norm_pair(3, av, 0)
    emit_norm_pair(3, av, 1)
    emit_outproj_group(3, 0)
    emit_outproj_group(3, 1)


def _build_nc():
    nc = bacc.Bacc("TRN2", target_bir_lowering=False, debug=False,
                   enable_asserts=False, num_devices=8)
    ins = {
        "xt": nc.dram_tensor("xt", [128, NCH * 8 * CH], BF16,
                             kind="ExternalInput").ap(),
        "wq": nc.dram_tensor("wq", [128, 8 * 512], BF16,
                             kind="ExternalInput").ap(),
        "wv": nc.dram_tensor("wv", [128, 8 * 256], BF16,
                             kind="ExternalInput").ap(),
        "wo": nc.dram_tensor("wo", [256, DM], F32, kind="ExternalInput").ap(),
        "cs": nc.dram_tensor("cs", [128, L], F32, kind="ExternalInput").ap(),
        "sn": nc.dram_tensor("sn", [128, L], F32, kind="ExternalInput").ap(),
    }
    outs = {"out": nc.dram_tensor("out", [L, DM], BF16, kind="ExternalOutput").ap()}
    with tile.TileContext(nc) as tc:
        _attn_kernel(tc, outs, ins)
    nc.compile()
    return nc


def _host_shard(X, token_positions, Wqkv, Wout):
    """Build the 8 per-core input maps."""
    X = np.asarray(X, dtype=np.float32)
    Wqkv = np.asarray(Wqkv, dtype=np.float32)
    Wout = np.asarray(Wout, dtype=np.float32)
    pos = np.asarray(token_positions)

    # Per-head dim order: [e0..e15, o0..o15, e16..e31, o16..o31] so the RoPE
    # partner (even<->odd of the same freq) is p^16 within a 32-quadrant.
    def head_order(base):
        return ([base + 2 * k for k in range(16)] +
                [base + 2 * k + 1 for k in range(16)] +
                [base + 2 * k for k in range(16, 32)] +
                [base + 2 * k + 1 for k in range(16, 32)])

    # RoPE tables in float32 arithmetic to mirror the f32 reference
    k = np.arange(DK // 2, dtype=np.float32)
    inv_freq = (np.float32(1.0) /
                np.power(np.float32(THETA), (np.float32(2.0) * k) / np.float32(DK)))
    inv_freq = inv_freq.astype(np.float32)
    # freq index per partition within a 64-block: [0:16]=f0..15, [16:32]=f0..15,
    # [32:48]=f16..31, [48:64]=f16..31; sign +1 on top blocks, -1 on bot blocks
    fidx = np.concatenate([np.arange(16), np.arange(16),
                           np.arange(16, 32), np.arange(16, 32)])
    sgn = np.concatenate([np.ones(16), -np.ones(16),
                          np.ones(16), -np.ones(16)]).astype(np.float32)
    fidx = np.tile(fidx, 2)           # 128 partitions (2 heads per tile)
    sgn = np.tile(sgn, 2)
    ang = (pos.astype(np.float32)[:, None, :] *
           inv_freq[fidx][None, :, None]).astype(np.float32)   # [B, 128, L]
    cs_all = np.cos(ang).astype(np.float32)
    sn_all = (np.sin(ang) * sgn[None, :, None]).astype(np.float32)

    in_maps = []
    for core in range(8):
        b, g = divmod(core, HPC)
        heads = [HPC * g + hh for hh in range(HPC)]
        q_rows, k_rows = [], []
        for h in heads:
            q_rows += head_order(DK * h)
            k_rows += head_order(DM + DK * h)
        # tile-major packs: [128, d-blocks * cols] so each group is one
        # contiguous DMA on the device side
        wq_c = (Wqkv[q_rows + k_rows, :].T.astype(ml_dtypes.bfloat16)
                .reshape(8, 128, 512).transpose(1, 0, 2).reshape(128, -1))
        v_rows = [2 * DM + DK * h + j for h in heads for j in range(DK)]
        wv_c = (Wqkv[v_rows, :].T.astype(ml_dtypes.bfloat16)
                .reshape(8, 128, 256).transpose(1, 0, 2).reshape(128, -1))
        wo_c = np.ascontiguousarray(Wout[:, 256 * g:256 * (g + 1)].T)
        xt_c = (X[b].T.astype(ml_dtypes.bfloat16)
                .reshape(8, 128, NCH, CH).transpose(1, 2, 0, 3)
                .reshape(128, -1))
        in_maps.append({
            "xt": np.ascontiguousarray(xt_c),
            "wq": np.ascontiguousarray(wq_c),
            "wv": np.ascontiguousarray(wv_c),
            "wo": wo_c,
            "cs": np.ascontiguousarray(cs_all[b]),
            "sn": np.ascontiguousarray(sn_all[b]),
        })
    return in_maps


def kernel(X, token_positions, Wqkv, Wout, _trace=False):
    if "nc" not in _cache:
        _cache["nc"] = _build_nc()
    nc = _cache["nc"]
    in_maps = _host_shard(X, token_positions, Wqkv, Wout)
    res = run_bass_kernel_spmd(nc, in_maps, list(range(8)), trace=_trace)
    _cache["last_results"] = res
    out = np.zeros((B, L, DM), dtype=np.float32)
    for core in range(8):
        out[core // HPC] += np.asarray(res.results[core]["out"],
                                       dtype=np.float32)
    return out
